# revision 13
# baseline (speedup 1.0000x reference)
"""Bass/Trainium2 kernel for nn_Encoder_Flows (4-layer SAGE encoder with
buggy prefix-mean aggregation), SPMD over 8 NeuronCores.

Math (per reference):
  x = flow_matrix.reshape(B*K, P)   # [32768, 1024]
  4x: out = agg @ w_l.T + b_l + x @ w_r.T ; out /= ||out||_row
  where agg[j] = mean_{i<j} x[i] for j < K=1024, else 0.
  final relu.

Strategy:
  - Shard the 32768 rows into 8 contiguous shards of 4096 (core c gets rows
    [4096c, 4096(c+1))). Rows >= 1024 are row-independent (agg = 0).
  - Feature-major on chip: activations live as A[d, cols]; matmuls are then
    always out[dout_tile, cols] = W_T_slice.T @ A with no transposes.
  - The prefix-mean for rows < 1024 (core 0 only) is a matmul against a
    lower-triangular coefficient matrix M (L[j,i] = 1/j, i<j):
      C = (G_rowmajor).T-contract against M.T, with G = x_k @ w_l.T computed
      row-major (its lhsT is exactly the feature-major activation tile).
    Cores 1-7 get M = 0 (same code, zero contribution).
  - fp16 matmul operands (full PE rate), fp32 PSUM accumulate, fp32 output.
  - Row norm (per free-dim column): square on ACT, column-sum via
    ones-vector matmul on PE, sqrt on ACT, reciprocal on DVE, partition
    broadcast via DMA, fused (psum+bias)*scale on DVE.
"""

import sys

if "/opt/trn_rl_repo" not in sys.path:
    sys.path.insert(0, "/opt/trn_rl_repo")

import numpy as np

B, K, P = 32, 1024, 1024
N_CORES = 8
RPC = (B * K) // N_CORES  # 4096 columns (rows of x) per core
CH = 512                  # chunk of columns processed at once
NCH = RPC // CH           # 8 chunks; chunks 0,1 hold the coupled rows 0..1023
DIMS = [(1024, 128), (128, 256), (256, 128), (128, 256)]
DOUT = DIMS[-1][1]

# nonzero [128i, 512j] blocks of M.T (MT[i,j] = 1/j if i<j else 0)
MT_BLOCKS = [(it, 0) for it in range(4)] + [(it, 1) for it in range(8)]


def _mt_block_id(it, jc):
    return it if jc == 0 else 4 + it


_CACHE = {}


def _build_program():
    import concourse.bass as bass
    import concourse.tile as tile
    from concourse import bacc, mybir

    f16 = mybir.dt.float16
    f32 = mybir.dt.float32
    AF = mybir.ActivationFunctionType
    OP = mybir.AluOpType

    nc = bacc.Bacc("TRN2", target_bir_lowering=False, debug=False)

    xt = nc.dram_tensor("xt", [P, RPC], f16, kind="ExternalInput").ap()
    mt = nc.dram_tensor("mt", [128, len(MT_BLOCKS) * CH], f16,
                        kind="ExternalInput").ap()
    wr_d, wl_d, b_d = [], [], []
    for li, (din, dout) in enumerate(DIMS):
        wr_d.append(nc.dram_tensor(f"wr{li}", [din, dout], f16,
                                   kind="ExternalInput").ap())
        wl_d.append(nc.dram_tensor(f"wl{li}", [din, dout], f16,
                                   kind="ExternalInput").ap())
        b_d.append(nc.dram_tensor(f"b{li}", [dout], f32,
                                  kind="ExternalInput").ap())
    out_d = nc.dram_tensor("out", [DOUT, RPC], f32, kind="ExternalOutput").ap()
    out_r = out_d.rearrange("(pt p) c -> p pt c", p=128)

    with tile.TileContext(nc) as tc:
        with (
            tc.tile_pool(name="consts", bufs=1) as consts,
            tc.tile_pool(name="xk", bufs=1) as xkp,
            tc.tile_pool(name="xs", bufs=6) as xsp,
            tc.tile_pool(name="pa", bufs=1) as pap,
            tc.tile_pool(name="ab", bufs=9) as abp,
            tc.tile_pool(name="gsb", bufs=8) as gsbp,
            tc.tile_pool(name="sq", bufs=8) as sqp,
            tc.tile_pool(name="snorm", bufs=8) as snp,
            tc.tile_pool(name="sbc", bufs=8) as sbcp,
            tc.tile_pool(name="ost", bufs=3) as ostp,
            tc.tile_pool(name="mainp", bufs=4, space="PSUM") as mainp,
            tc.tile_pool(name="ssp", bufs=2, space="PSUM") as sspp,
            tc.tile_pool(name="gp", bufs=2, space="PSUM") as gpp,
        ):
            # ---- constants ----
            wr_sb, wl_sb, b_sb = [], [], []
            for li, (din, dout) in enumerate(DIMS):
                kt = din // 128
                w1 = consts.tile([128, kt, dout], f16, tag=f"wr{li}")
                nc.sync.dma_start(
                    out=w1, in_=wr_d[li].rearrange("(k p) d -> p k d", p=128))
                wr_sb.append(w1)
                w2 = consts.tile([128, kt, dout], f16, tag=f"wl{li}")
                nc.sync.dma_start(
                    out=w2, in_=wl_d[li].rearrange("(k p) d -> p k d", p=128))
                wl_sb.append(w2)
                pt = dout // 128
                bt = consts.tile([128, pt], f32, tag=f"b{li}")
                nc.sync.dma_start(
                    out=bt, in_=b_d[li].rearrange("(pt p) -> p pt", p=128))
                b_sb.append(bt)
            mt_sb = consts.tile([128, len(MT_BLOCKS), CH], f16, tag="mt")
            nc.sync.dma_start(
                out=mt_sb,
                in_=mt.rearrange("p (b c) -> p b c", b=len(MT_BLOCKS)))
            ones_col = consts.tile([128, 1], f16, tag="ones_col")
            nc.vector.memset(ones_col, 1.0)

            # phase-A input: x.T columns 0..1023, kept resident
            xk_sb = xkp.tile([128, P // 128, K], f16, tag="xk")
            nc.sync.dma_start(
                out=xk_sb,
                in_=xt.rearrange("(k p) c -> p k c", p=128)[:, :, 0:K])

            # persistent phase-A activations per layer (columns 0..1023)
            pa_sb = []
            for li, (din, dout) in enumerate(DIMS[:-1]):
                pa_sb.append(pap.tile([128, dout // 128, K], f16,
                                      tag=f"pa{li}", name=f"pa{li}"))

            def epilogue(li, mains, aout, aout_sl, is_last):
                """mains: list of Pt main psum tiles (pre-bias).
                aout[aout_sl] <- normalized fp16 (or fp32+relu out)."""
                dout = DIMS[li][1]
                ptn = dout // 128
                ss = sspp.tile([1, CH], f32, tag="ss")
                sqs = []
                for pt in range(ptn):
                    sq = sqp.tile([128, CH], f16, tag="sq")
                    # sq = (main + b)^2  on ACT, psum -> sbuf fp16
                    nc.scalar.activation(out=sq, in_=mains[pt], func=AF.Square,
                                         bias=b_sb[li][:, pt:pt + 1], scale=1.0)
                    sqs.append(sq)
                for pt in range(ptn):
                    nc.tensor.matmul(ss, lhsT=ones_col, rhs=sqs[pt],
                                     start=(pt == 0), stop=(pt == ptn - 1))
                # sqrt fused into the PSUM->SBUF copy on ACT. reciprocal on
                # a [1, CH] strip is catastrophically slow on DVE (single
                # partition = single lane), so fold to [128, CH/128] via
                # tiny DMAs and run it on all 128 lanes.
                # The fold/unfold DMAs are issued from the engines that sit
                # at the same point of the dependency chain (ACT after its
                # sqrt, DVE after its reciprocal) — putting them on the
                # shared sync queue head-of-line blocks unrelated loads.
                nrm = snp.tile([1, CH], f32, tag="nrm")
                nc.scalar.activation(out=nrm, in_=ss, func=AF.Sqrt)
                fold = snp.tile([128, CH // 128], f32, tag="fold")
                nc.scalar.dma_start(out=fold, in_=nrm)
                s4 = snp.tile([128, CH // 128], f16, tag="s4")
                with nc.allow_low_precision(reason="1/norm rounds to fp16"):
                    nc.vector.reciprocal(out=s4, in_=fold)
                s16 = snp.tile([1, CH], f16, tag="s16")
                nc.gpsimd.dma_start(out=s16, in_=s4)
                # partition-broadcast on GpSimd (otherwise-idle engine)
                sb = sbcp.tile([128, CH], f16, tag="sbc")
                nc.gpsimd.partition_broadcast(sb, s16)
                if not is_last:
                    for pt in range(ptn):
                        nc.vector.scalar_tensor_tensor(
                            out=aout[:, pt, aout_sl], in0=mains[pt],
                            scalar=b_sb[li][:, pt:pt + 1], in1=sb,
                            op0=OP.add, op1=OP.mult)
                else:
                    ost = ostp.tile([128, ptn, CH], f32, tag="ost")
                    for pt in range(ptn):
                        rl = sqp.tile([128, CH], f16, tag="rl")
                        nc.scalar.activation(out=rl, in_=mains[pt],
                                             func=AF.Relu,
                                             bias=b_sb[li][:, pt:pt + 1],
                                             scale=1.0)
                        nc.vector.tensor_mul(out=ost[:, pt, :], in0=rl, in1=sb)
                    nc.sync.dma_start(out=out_r[:, :, aout_sl], in_=ost)

            # ---------------- phase A: columns 0..1023 (coupled) ----------
            for li, (din, dout) in enumerate(DIMS):
                ktn = din // 128
                ptn = dout // 128
                ain = xk_sb if li == 0 else pa_sb[li - 1]
                # G[i, f] = x_k @ w_l.T, row(i)-major: 8 i-tiles
                g_sb = []
                for it in range(8):
                    gp = gpp.tile([128, dout], f32, tag="gp")
                    for kt in range(ktn):
                        nc.tensor.matmul(
                            gp, lhsT=ain[:, kt, it * 128:(it + 1) * 128],
                            rhs=wl_sb[li][:, kt, :],
                            start=(kt == 0), stop=(kt == ktn - 1))
                    g = gsbp.tile([128, dout], f16, tag="g")
                    nc.scalar.copy(g, gp)
                    g_sb.append(g)
                for jc in range(2):
                    its = [it for (it, j) in MT_BLOCKS if j == jc]
                    mains = []
                    for pt in range(ptn):
                        mp = mainp.tile([128, CH], f32, tag="mp")
                        for kt in range(ktn):
                            nc.tensor.matmul(
                                mp,
                                lhsT=wr_sb[li][:, kt, pt * 128:(pt + 1) * 128],
                                rhs=ain[:, kt, jc * CH:(jc + 1) * CH],
                                start=(kt == 0), stop=False)
                        for ii, it in enumerate(its):
                            nc.tensor.matmul(
                                mp,
                                lhsT=g_sb[it][:, pt * 128:(pt + 1) * 128],
                                rhs=mt_sb[:, _mt_block_id(it, jc), :],
                                start=False, stop=(ii == len(its) - 1))
                        mains.append(mp)
                    sl = slice(jc * CH, (jc + 1) * CH)
                    if li < 3:
                        epilogue(li, mains, pa_sb[li], sl, False)
                    else:
                        epilogue(li, mains, None, sl, True)

            # ---------------- phase B: columns 1024..4095 (plain) ---------
            for ch in range(2, NCH):
                xs = xsp.tile([128, P // 128, CH], f16, tag="xs")
                nc.sync.dma_start(
                    out=xs,
                    in_=xt.rearrange("(k p) c -> p k c",
                                     p=128)[:, :, ch * CH:(ch + 1) * CH])
                ain = xs
                for li, (din, dout) in enumerate(DIMS):
                    ktn = din // 128
                    ptn = dout // 128
                    mains = []
                    for pt in range(ptn):
                        mp = mainp.tile([128, CH], f32, tag="mp")
                        for kt in range(ktn):
                            nc.tensor.matmul(
                                mp,
                                lhsT=wr_sb[li][:, kt, pt * 128:(pt + 1) * 128],
                                rhs=ain[:, kt, :],
                                start=(kt == 0), stop=(kt == ktn - 1))
                        mains.append(mp)
                    if li < 3:
                        anext = abp.tile([128, ptn, CH], f16, tag="ab")
                        epilogue(li, mains, anext, slice(0, CH), False)
                        ain = anext
                    else:
                        epilogue(li, mains, None,
                                 slice(ch * CH, (ch + 1) * CH), True)

    nc.compile()
    return nc


def _prep_inputs(flow_matrix, ws):
    """ws: list of (w_l, b_l, w_r) fp32. Returns list of 8 in_maps."""
    x = np.ascontiguousarray(flow_matrix.reshape(B * K, P))
    xt_full = np.ascontiguousarray(x.T.astype(np.float16))  # [P, 32768]

    # M.T packed nonzero blocks, fp16
    inv = np.zeros(K, np.float32)
    inv[1:] = 1.0 / np.arange(1, K, dtype=np.float32)
    mt_packed = np.zeros((128, len(MT_BLOCKS) * CH), np.float16)
    for bid, (it, jc) in enumerate(MT_BLOCKS):
        i0, j0 = it * 128, jc * CH
        blk = np.zeros((128, CH), np.float32)
        for pp in range(128):
            i = i0 + pp
            jj = np.arange(j0, j0 + CH)
            blk[pp] = np.where(jj > i, inv[jj], 0.0)
        mt_packed[:, bid * CH:(bid + 1) * CH] = blk.astype(np.float16)
    mt_zero = np.zeros_like(mt_packed)

    base = {}
    for li, (w_l, b_l, w_r) in enumerate(ws):
        base[f"wr{li}"] = np.ascontiguousarray(w_r.T.astype(np.float16))
        base[f"wl{li}"] = np.ascontiguousarray(w_l.T.astype(np.float16))
        base[f"b{li}"] = np.ascontiguousarray(b_l.astype(np.float32))

    in_maps = []
    for c in range(N_CORES):
        m = dict(base)
        m["xt"] = np.ascontiguousarray(xt_full[:, c * RPC:(c + 1) * RPC])
        m["mt"] = mt_packed if c == 0 else mt_zero
        in_maps.append(m)
    return in_maps


def kernel(flow_matrix, w_l1, b_l1, w_r1, w_l2, b_l2, w_r2,
           w_l3, b_l3, w_r3, w_l4, b_l4, w_r4, _trace=False, _tmpdir=None):
    from concourse import bass_utils

    flow_matrix = np.asarray(flow_matrix, dtype=np.float32)
    ws = [(np.asarray(w_l1, np.float32), np.asarray(b_l1, np.float32),
           np.asarray(w_r1, np.float32)),
          (np.asarray(w_l2, np.float32), np.asarray(b_l2, np.float32),
           np.asarray(w_r2, np.float32)),
          (np.asarray(w_l3, np.float32), np.asarray(b_l3, np.float32),
           np.asarray(w_r3, np.float32)),
          (np.asarray(w_l4, np.float32), np.asarray(b_l4, np.float32),
           np.asarray(w_r4, np.float32))]

    if "nc" not in _CACHE:
        _CACHE["nc"] = _build_program()
    nc = _CACHE["nc"]

    in_maps = _prep_inputs(flow_matrix, ws)
    res = bass_utils.run_bass_kernel_spmd(
        nc, in_maps, core_ids=list(range(N_CORES)), trace=_trace,
        tmpdir=_tmpdir)

    y = np.empty((B * K, DOUT), np.float32)
    for c in range(N_CORES):
        out_c = res.results[c]["out"]  # [DOUT, RPC] fp32
        y[c * RPC:(c + 1) * RPC, :] = out_c.T
    _CACHE["last_exec_time_ns"] = res.exec_time_ns
    return (y.reshape(B, K, DOUT), 1)


if __name__ == "__main__":
    rng = np.random.default_rng(0)
    fm = rng.standard_normal((B, K, P)).astype(np.float32)
    args = []
    for (din, dout) in DIMS:
        s = 1.0 / np.sqrt(din)
        args += [rng.uniform(-s, s, (dout, din)).astype(np.float32),
                 rng.uniform(-s, s, dout).astype(np.float32),
                 rng.uniform(-s, s, (dout, din)).astype(np.float32)]
    y, _ = kernel(fm, *args)
    print("ok", y.shape, y.dtype)


# revision 16
# speedup vs baseline: 2.0203x; 2.0203x over previous
"""Bass/Trainium2 kernel for nn_Encoder_Flows (4-layer SAGE encoder with
buggy prefix-mean aggregation), SPMD over 8 NeuronCores.

Math (per reference):
  x = flow_matrix.reshape(B*K, P)   # [32768, 1024]
  4x: out = agg @ w_l.T + b_l + x @ w_r.T ; out /= ||out||_row
  where agg[j] = mean_{i<j} x[i] for j < K=1024, else 0.
  final relu.

Strategy:
  - Shard the 32768 rows into 8 contiguous shards of 4096 (core c gets rows
    [4096c, 4096(c+1))). Rows >= 1024 are row-independent (agg = 0).
  - Feature-major on chip: activations live as A[d, cols]; matmuls are then
    always out[dout_tile, cols] = W_T_slice.T @ A with no transposes.
  - The prefix-mean for rows < 1024 (core 0 only) is a matmul against a
    lower-triangular coefficient matrix M (L[j,i] = 1/j, i<j):
      C = G contracted against M.T, with G = x_k @ w_l.T computed row-major
      (its lhsT is exactly the feature-major activation tile).
    Cores 1-7 get M = 0 (same SPMD code, zero contribution).
  - fp16 matmul operands (full PE rate), fp32 PSUM accumulate, fp32 output.
  - All 8 column-chunks march through the layers together, stage by stage
    (the Tile scheduler follows emission order per engine, so emission
    interleaving IS the software pipeline).
  - Row norm (per free-dim column): bias+copy to fp16 (frees PSUM fast),
    square on DVE, column-sum via ones-vector matmul on PE, sqrt on ACT,
    reciprocal on DVE on a [128, CH/128] refold (a [1, CH] strip would be
    single-lane), partition-broadcast on GpSimd, fused multiply on DVE.
"""

import sys

if "/opt/trn_rl_repo" not in sys.path:
    sys.path.insert(0, "/opt/trn_rl_repo")

import numpy as np

B, K, P = 32, 1024, 1024
N_CORES = 8
RPC = (B * K) // N_CORES  # 4096 columns (rows of x) per core
CH = 512                  # chunk of columns processed at once
NCH = RPC // CH           # 8 chunks; chunks 0,1 hold the coupled rows 0..1023
DIMS = [(1024, 128), (128, 256), (256, 128), (128, 256)]
DOUT = DIMS[-1][1]

# nonzero [128i, 512j] blocks of M.T (MT[i,j] = 1/j if i<j else 0)
MT_BLOCKS = [(it, 0) for it in range(4)] + [(it, 1) for it in range(8)]


def _mt_block_id(it, jc):
    return it if jc == 0 else 4 + it


_CACHE = {}


def _build_program():
    import concourse.bass as bass  # noqa: F401
    import concourse.tile as tile
    from concourse import bacc, mybir

    f16 = mybir.dt.float16
    f32 = mybir.dt.float32
    AF = mybir.ActivationFunctionType
    OP = mybir.AluOpType

    nc = bacc.Bacc("TRN2", target_bir_lowering=False, debug=False)

    xt = nc.dram_tensor("xt", [P, RPC], f16, kind="ExternalInput").ap()
    mt = nc.dram_tensor("mt", [128, len(MT_BLOCKS) * CH], f16,
                        kind="ExternalInput").ap()
    wr_d, wl_d, b_d = [], [], []
    for li, (din, dout) in enumerate(DIMS):
        wr_d.append(nc.dram_tensor(f"wr{li}", [din, dout], f16,
                                   kind="ExternalInput").ap())
        wl_d.append(nc.dram_tensor(f"wl{li}", [din, dout], f16,
                                   kind="ExternalInput").ap())
        b_d.append(nc.dram_tensor(f"b{li}", [dout], f32,
                                  kind="ExternalInput").ap())
    out_d = nc.dram_tensor("out", [DOUT, RPC], f32, kind="ExternalOutput").ap()
    out_r = out_d.rearrange("(pt p) c -> p pt c", p=128)

    with tile.TileContext(nc) as tc:
        with (
            tc.tile_pool(name="consts", bufs=1) as consts,
            tc.tile_pool(name="xk", bufs=1) as xkp,
            tc.tile_pool(name="xs", bufs=6) as xsp,
            tc.tile_pool(name="pa", bufs=1) as pap,
            tc.tile_pool(name="ab", bufs=13) as abp,
            tc.tile_pool(name="raw", bufs=8) as rawp,
            tc.tile_pool(name="gsb", bufs=8) as gsbp,
            tc.tile_pool(name="sq", bufs=8) as sqp,
            tc.tile_pool(name="snorm", bufs=8) as snp,
            tc.tile_pool(name="sbc", bufs=8) as sbcp,
            tc.tile_pool(name="ost", bufs=3) as ostp,
            tc.tile_pool(name="mainp", bufs=4, space="PSUM") as mainp,
            tc.tile_pool(name="ssp", bufs=2, space="PSUM") as sspp,
            tc.tile_pool(name="gp", bufs=2, space="PSUM") as gpp,
        ):
            # ---- constants ----
            wr_sb, wl_sb, b_sb = [], [], []
            for li, (din, dout) in enumerate(DIMS):
                kt = din // 128
                w1 = consts.tile([128, kt, dout], f16, tag=f"wr{li}")
                nc.sync.dma_start(
                    out=w1, in_=wr_d[li].rearrange("(k p) d -> p k d", p=128))
                wr_sb.append(w1)
                w2 = consts.tile([128, kt, dout], f16, tag=f"wl{li}")
                nc.sync.dma_start(
                    out=w2, in_=wl_d[li].rearrange("(k p) d -> p k d", p=128))
                wl_sb.append(w2)
                pt = dout // 128
                bt = consts.tile([128, pt], f32, tag=f"b{li}")
                nc.sync.dma_start(
                    out=bt, in_=b_d[li].rearrange("(pt p) -> p pt", p=128))
                b_sb.append(bt)
            mt_sb = consts.tile([128, len(MT_BLOCKS), CH], f16, tag="mt")
            nc.sync.dma_start(
                out=mt_sb,
                in_=mt.rearrange("p (b c) -> p b c", b=len(MT_BLOCKS)))
            ones_col = consts.tile([128, 1], f16, tag="ones_col")
            nc.vector.memset(ones_col, 1.0)

            # coupled input: x.T columns 0..1023, kept resident
            xk_sb = xkp.tile([128, P // 128, K], f16, tag="xk")
            nc.sync.dma_start(
                out=xk_sb,
                in_=xt.rearrange("(k p) c -> p k c", p=128)[:, :, 0:K])
            # streamed input for the plain chunks
            xs_sb = {}
            for ch in range(2, NCH):
                x1 = xsp.tile([128, P // 128, CH], f16, tag="xs",
                              name=f"xs{ch}")
                nc.sync.dma_start(
                    out=x1,
                    in_=xt.rearrange("(k p) c -> p k c",
                                     p=128)[:, :, ch * CH:(ch + 1) * CH])
                xs_sb[ch] = x1

            # persistent coupled activations per layer (columns 0..1023)
            pa_sb = []
            for li, (din, dout) in enumerate(DIMS[:-1]):
                pa_sb.append(pap.tile([128, dout // 128, K], f16,
                                      tag=f"pa{li}", name=f"pa{li}"))

            # per-chunk current activation APs: [128, kt, CH] views
            ain = {}
            for ch in range(NCH):
                if ch < 2:
                    ain[ch] = xk_sb[:, :, ch * CH:(ch + 1) * CH]
                else:
                    ain[ch] = xs_sb[ch]

            for li, (din, dout) in enumerate(DIMS):
                ktn = din // 128
                ptn = dout // 128
                is_last = li == 3
                gain = xk_sb if li == 0 else pa_sb[li - 1]

                # --- coupled G: G[i, f] = x_k @ w_l.T, row(i)-major ---
                g_sb = []
                for it in range(8):
                    gp = gpp.tile([128, dout], f32, tag="gp")
                    for kt in range(ktn):
                        nc.tensor.matmul(
                            gp, lhsT=gain[:, kt, it * 128:(it + 1) * 128],
                            rhs=wl_sb[li][:, kt, :],
                            start=(kt == 0), stop=(kt == ktn - 1))
                    g = gsbp.tile([128, dout], f16, tag="g")
                    nc.scalar.copy(g, gp)
                    g_sb.append(g)

                # --- S0: mains (+ C' for coupled chunks) ---
                mains = {}
                for ch in range(NCH):
                    for pt in range(ptn):
                        mp = mainp.tile([128, CH], f32, tag="mp",
                                        name=f"mp{ch}_{pt}")
                        for kt in range(ktn):
                            nc.tensor.matmul(
                                mp,
                                lhsT=wr_sb[li][:, kt, pt * 128:(pt + 1) * 128],
                                rhs=ain[ch][:, kt, :],
                                start=(kt == 0),
                                stop=(kt == ktn - 1 and ch >= 2))
                        if ch < 2:
                            its = [it for (it, j) in MT_BLOCKS if j == ch]
                            for ii, it in enumerate(its):
                                nc.tensor.matmul(
                                    mp,
                                    lhsT=g_sb[it][:, pt * 128:(pt + 1) * 128],
                                    rhs=mt_sb[:, _mt_block_id(it, ch), :],
                                    start=False, stop=(ii == len(its) - 1))
                        mains[(ch, pt)] = mp

                    # S1 immediately per chunk: raw16 = main + b (frees PSUM).
                    # Alternate ACT/DVE to balance engine load.
                    raw = rawp.tile([128, ptn, CH], f16, tag="raw",
                                    name=f"raw{ch}")
                    for pt in range(ptn):
                        if pt % 2 == 0:
                            nc.scalar.activation(
                                out=raw[:, pt, :], in_=mains[(ch, pt)],
                                func=AF.Identity,
                                bias=b_sb[li][:, pt:pt + 1], scale=1.0)
                        else:
                            nc.vector.tensor_scalar_add(
                                out=raw[:, pt, :], in0=mains[(ch, pt)],
                                scalar1=b_sb[li][:, pt:pt + 1])
                    mains[ch] = raw

                # --- S2: sq = (raw+b)^2 on DVE; S3: ss += ones.T @ sq ---
                sss = {}
                sqs = {}
                for ch in range(NCH):
                    raw = mains[ch]
                    sq = sqp.tile([128, ptn, CH], f16, tag="sq",
                                  name=f"sq{ch}")
                    for pt in range(ptn):
                        nc.vector.tensor_mul(
                            out=sq[:, pt, :], in0=raw[:, pt, :],
                            in1=raw[:, pt, :])
                    sqs[ch] = sq
                for ch in range(NCH):
                    ss = sspp.tile([1, CH], f32, tag="ss", name=f"ss{ch}")
                    for pt in range(ptn):
                        nc.tensor.matmul(ss, lhsT=ones_col,
                                         rhs=sqs[ch][:, pt, :],
                                         start=(pt == 0), stop=(pt == ptn - 1))
                    sss[ch] = ss

                # --- S4..S6: sqrt (ACT, psum->sbuf), fold, recip, bcast ---
                sbs = {}
                for ch in range(NCH):
                    nrm = snp.tile([1, CH], f32, tag="nrm", name=f"nrm{ch}")
                    nc.scalar.activation(out=nrm, in_=sss[ch], func=AF.Sqrt)
                    fold = snp.tile([128, CH // 128], f32, tag="fold",
                                    name=f"fold{ch}")
                    nc.scalar.dma_start(out=fold, in_=nrm)
                    s4 = snp.tile([128, CH // 128], f16, tag="s4",
                                  name=f"s4{ch}")
                    with nc.allow_low_precision(reason="1/norm to fp16"):
                        nc.vector.reciprocal(out=s4, in_=fold)
                    s16 = snp.tile([1, CH], f16, tag="s16", name=f"s16{ch}")
                    nc.gpsimd.dma_start(out=s16, in_=s4)
                    sb = sbcp.tile([128, CH], f16, tag="sbc", name=f"sb{ch}")
                    nc.gpsimd.partition_broadcast(sb, s16)
                    sbs[ch] = sb

                # --- S7: apply scale (and bias where still missing) ---
                for ch in range(NCH):
                    raw = mains[ch]
                    sb = sbs[ch]
                    if not is_last:
                        if ch < 2:
                            aout = pa_sb[li]
                            asl = (slice(None), slice(None),
                                   slice(ch * CH, (ch + 1) * CH))
                        else:
                            anext = abp.tile([128, ptn, CH], f16, tag="ab",
                                             name=f"ab{ch}")
                            aout = anext
                            asl = (slice(None), slice(None), slice(0, CH))
                            ain[ch] = anext
                        for pt in range(ptn):
                            dst = aout[asl[0], pt, asl[2]]
                            nc.vector.tensor_mul(
                                out=dst, in0=raw[:, pt, :], in1=sb)
                        if ch < 2:
                            ain[ch] = pa_sb[li][:, :,
                                               ch * CH:(ch + 1) * CH]
                    else:
                        ost = ostp.tile([128, ptn, CH], f32, tag="ost",
                                        name=f"ost{ch}")
                        for pt in range(ptn):
                            # relu((raw+b)*s) = max(raw+b,0)*s since s>0
                            nc.vector.scalar_tensor_tensor(
                                out=ost[:, pt, :], in0=raw[:, pt, :],
                                scalar=0.0, in1=sb, op0=OP.max, op1=OP.mult)
                        nc.sync.dma_start(
                            out=out_r[:, :, ch * CH:(ch + 1) * CH], in_=ost)

    nc.compile()
    return nc


def _prep_inputs(flow_matrix, ws):
    """ws: list of (w_l, b_l, w_r) fp32. Returns list of 8 in_maps."""
    x = np.ascontiguousarray(flow_matrix.reshape(B * K, P))
    xt_full = np.ascontiguousarray(x.T.astype(np.float16))  # [P, 32768]

    # M.T packed nonzero blocks, fp16
    inv = np.zeros(K, np.float32)
    inv[1:] = 1.0 / np.arange(1, K, dtype=np.float32)
    mt_packed = np.zeros((128, len(MT_BLOCKS) * CH), np.float16)
    for bid, (it, jc) in enumerate(MT_BLOCKS):
        i0, j0 = it * 128, jc * CH
        blk = np.zeros((128, CH), np.float32)
        for pp in range(128):
            i = i0 + pp
            jj = np.arange(j0, j0 + CH)
            blk[pp] = np.where(jj > i, inv[jj], 0.0)
        mt_packed[:, bid * CH:(bid + 1) * CH] = blk.astype(np.float16)
    mt_zero = np.zeros_like(mt_packed)

    base = {}
    for li, (w_l, b_l, w_r) in enumerate(ws):
        base[f"wr{li}"] = np.ascontiguousarray(w_r.T.astype(np.float16))
        base[f"wl{li}"] = np.ascontiguousarray(w_l.T.astype(np.float16))
        base[f"b{li}"] = np.ascontiguousarray(b_l.astype(np.float32))

    in_maps = []
    for c in range(N_CORES):
        m = dict(base)
        m["xt"] = np.ascontiguousarray(xt_full[:, c * RPC:(c + 1) * RPC])
        m["mt"] = mt_packed if c == 0 else mt_zero
        in_maps.append(m)
    return in_maps


def kernel(flow_matrix, w_l1, b_l1, w_r1, w_l2, b_l2, w_r2,
           w_l3, b_l3, w_r3, w_l4, b_l4, w_r4, _trace=False, _tmpdir=None):
    from concourse import bass_utils

    flow_matrix = np.asarray(flow_matrix, dtype=np.float32)
    ws = [(np.asarray(w_l1, np.float32), np.asarray(b_l1, np.float32),
           np.asarray(w_r1, np.float32)),
          (np.asarray(w_l2, np.float32), np.asarray(b_l2, np.float32),
           np.asarray(w_r2, np.float32)),
          (np.asarray(w_l3, np.float32), np.asarray(b_l3, np.float32),
           np.asarray(w_r3, np.float32)),
          (np.asarray(w_l4, np.float32), np.asarray(b_l4, np.float32),
           np.asarray(w_r4, np.float32))]

    if "nc" not in _CACHE:
        _CACHE["nc"] = _build_program()
    nc = _CACHE["nc"]

    in_maps = _prep_inputs(flow_matrix, ws)
    res = bass_utils.run_bass_kernel_spmd(
        nc, in_maps, core_ids=list(range(N_CORES)), trace=_trace,
        tmpdir=_tmpdir)

    y = np.empty((B * K, DOUT), np.float32)
    for c in range(N_CORES):
        out_c = res.results[c]["out"]  # [DOUT, RPC] fp32
        y[c * RPC:(c + 1) * RPC, :] = out_c.T
    _CACHE["last_exec_time_ns"] = res.exec_time_ns
    return (y.reshape(B, K, DOUT), 1)


if __name__ == "__main__":
    rng = np.random.default_rng(0)
    fm = rng.standard_normal((B, K, P)).astype(np.float32)
    args = []
    for (din, dout) in DIMS:
        s = 1.0 / np.sqrt(din)
        args += [rng.uniform(-s, s, (dout, din)).astype(np.float32),
                 rng.uniform(-s, s, dout).astype(np.float32),
                 rng.uniform(-s, s, (dout, din)).astype(np.float32)]
    y, _ = kernel(fm, *args)
    print("ok", y.shape, y.dtype)


# revision 21
# speedup vs baseline: 2.8818x; 1.4264x over previous
"""Bass/Trainium2 kernel for nn_Encoder_Flows (4-layer SAGE encoder with
buggy prefix-mean aggregation), SPMD over 8 NeuronCores.

Math (per reference):
  x = flow_matrix.reshape(B*K, P)   # [32768, 1024]
  4x: out = agg @ w_l.T + b_l + x @ w_r.T ; out /= ||out||_row
  where agg[j] = mean_{i<j} x[i] for j < K=1024, else 0.
  final relu.

Strategy:
  - Shard the 32768 rows into 8 contiguous shards of 4096 (core c gets rows
    [4096c, 4096(c+1))). Rows >= 1024 are row-independent (agg = 0).
  - Feature-major on chip: activations live as A[d, cols]; matmuls are then
    always out[dout_tile, cols] = W_T_slice.T @ A with no transposes.
  - The prefix-mean for rows < 1024 (core 0 only) is a matmul against a
    lower-triangular coefficient matrix M (L[j,i] = 1/j, i<j):
      C = G contracted against M.T, with G = x_k @ w_l.T computed row-major
      (its lhsT is exactly the feature-major activation tile).
    Cores 1-7 get M = 0 (same SPMD code, zero contribution).
  - fp16 matmul operands (full PE rate), fp32 PSUM accumulate, fp32 output.
  - All 8 column-chunks march through the layers together, stage by stage
    (the Tile scheduler follows emission order per engine, so emission
    interleaving IS the software pipeline).
  - Row norm (per free-dim column): bias+copy to fp16 (frees PSUM fast),
    square on DVE, column-sum via ones-vector matmul on PE, sqrt on ACT,
    reciprocal on DVE on a [128, CH/128] refold (a [1, CH] strip would be
    single-lane), partition-broadcast on GpSimd, fused multiply on DVE.
"""

import sys

if "/opt/trn_rl_repo" not in sys.path:
    sys.path.insert(0, "/opt/trn_rl_repo")

import numpy as np

B, K, P = 32, 1024, 1024
N_CORES = 8
RPC = (B * K) // N_CORES  # 4096 columns (rows of x) per core
CH = 512                  # chunk of columns processed at once
NCH = RPC // CH           # 8 chunks; chunks 0,1 hold the coupled rows 0..1023
DIMS = [(1024, 128), (128, 256), (256, 128), (128, 256)]
DOUT = DIMS[-1][1]

# nonzero [128i, 512j] blocks of M.T (MT[i,j] = 1/j if i<j else 0)
MT_BLOCKS = [(it, 0) for it in range(4)] + [(it, 1) for it in range(8)]


def _mt_block_id(it, jc):
    return it if jc == 0 else 4 + it


_CACHE = {}


def _build_program():
    import concourse.bass as bass  # noqa: F401
    import concourse.tile as tile
    from concourse import bacc, mybir

    f16 = mybir.dt.float16
    f32 = mybir.dt.float32
    AF = mybir.ActivationFunctionType
    OP = mybir.AluOpType

    nc = bacc.Bacc("TRN2", target_bir_lowering=False, debug=False)

    xt = nc.dram_tensor("xt", [P, RPC], f16, kind="ExternalInput").ap()
    mt = nc.dram_tensor("mt", [128, len(MT_BLOCKS) * CH], f16,
                        kind="ExternalInput").ap()
    wr_d, wl_d, b_d = [], [], []
    for li, (din, dout) in enumerate(DIMS):
        kt, pt = din // 128, dout // 128
        # host pre-packs weights partition-major so each load is one
        # contiguous row per partition
        wr_d.append(nc.dram_tensor(f"wr{li}", [128, kt * dout], f16,
                                   kind="ExternalInput").ap())
        wl_d.append(nc.dram_tensor(f"wl{li}", [128, kt * dout], f16,
                                   kind="ExternalInput").ap())
        b_d.append(nc.dram_tensor(f"b{li}", [128, pt], f32,
                                  kind="ExternalInput").ap())
    out_d = nc.dram_tensor("out", [DOUT, RPC], f32, kind="ExternalOutput").ap()
    out_r = out_d.rearrange("(pt p) c -> p pt c", p=128)

    with tile.TileContext(nc) as tc:
        with (
            tc.tile_pool(name="consts", bufs=1) as consts,
            tc.tile_pool(name="xk", bufs=1) as xkp,
            tc.tile_pool(name="xs", bufs=6) as xsp,
            tc.tile_pool(name="pa", bufs=1) as pap,
            tc.tile_pool(name="ab", bufs=13) as abp,
            tc.tile_pool(name="raw", bufs=8) as rawp,
            tc.tile_pool(name="gsb", bufs=8) as gsbp,
            tc.tile_pool(name="sq", bufs=8) as sqp,
            tc.tile_pool(name="sbc", bufs=8) as sbcp,
            tc.tile_pool(name="ost", bufs=3) as ostp,
            tc.tile_pool(name="mainp", bufs=4, space="PSUM") as mainp,
            tc.tile_pool(name="ssp", bufs=2, space="PSUM") as sspp,
            tc.tile_pool(name="gp", bufs=2, space="PSUM") as gpp,
        ):
            # coupled input first: x.T columns 0..1023 — the first matmuls
            # depend on it, so it must not queue behind other loads
            xk_sb = xkp.tile([128, P // 128, K], f16, tag="xk")
            nc.sync.dma_start(
                out=xk_sb,
                in_=xt.rearrange("(k p) c -> p k c", p=128)[:, :, 0:K])

            # ---- constants ----
            wr_sb, wl_sb, b_sb = [], [], []
            for li, (din, dout) in enumerate(DIMS):
                kt = din // 128
                w1 = consts.tile([128, kt, dout], f16, tag=f"wr{li}")
                nc.sync.dma_start(
                    out=w1, in_=wr_d[li].rearrange("p (k d) -> p k d", k=kt))
                wr_sb.append(w1)
                w2 = consts.tile([128, kt, dout], f16, tag=f"wl{li}")
                nc.sync.dma_start(
                    out=w2, in_=wl_d[li].rearrange("p (k d) -> p k d", k=kt))
                wl_sb.append(w2)
                pt = dout // 128
                bt = consts.tile([128, pt], f32, tag=f"b{li}")
                nc.sync.dma_start(out=bt, in_=b_d[li])
                b_sb.append(bt)
            mt_sb = consts.tile([128, len(MT_BLOCKS), CH], f16, tag="mt")
            nc.sync.dma_start(
                out=mt_sb,
                in_=mt.rearrange("p (b c) -> p b c", b=len(MT_BLOCKS)))
            # all-ones stationary: the sumsq matmul then sums over features
            # AND broadcasts the result to every partition in one op
            ones128 = consts.tile([128, 128], f16, tag="ones128")
            nc.vector.memset(ones128, 1.0)
            # streamed input for the plain chunks
            xs_sb = {}
            for ch in range(2, NCH):
                x1 = xsp.tile([128, P // 128, CH], f16, tag="xs",
                              name=f"xs{ch}")
                nc.sync.dma_start(
                    out=x1,
                    in_=xt.rearrange("(k p) c -> p k c",
                                     p=128)[:, :, ch * CH:(ch + 1) * CH])
                xs_sb[ch] = x1

            # persistent coupled activations per layer (columns 0..1023)
            pa_sb = []
            for li, (din, dout) in enumerate(DIMS[:-1]):
                pa_sb.append(pap.tile([128, dout // 128, K], f16,
                                      tag=f"pa{li}", name=f"pa{li}"))

            # per-chunk current activation APs: [128, kt, CH] views
            ain = {}
            for ch in range(NCH):
                if ch < 2:
                    ain[ch] = xk_sb[:, :, ch * CH:(ch + 1) * CH]
                else:
                    ain[ch] = xs_sb[ch]

            for li, (din, dout) in enumerate(DIMS):
                ktn = din // 128
                ptn = dout // 128
                is_last = li == 3
                gain = xk_sb if li == 0 else pa_sb[li - 1]

                # --- coupled G: G[i, f] = x_k @ w_l.T, row(i)-major ---
                g_sb = []
                for it in range(8):
                    gp = gpp.tile([128, dout], f32, tag="gp")
                    for kt in range(ktn):
                        nc.tensor.matmul(
                            gp, lhsT=gain[:, kt, it * 128:(it + 1) * 128],
                            rhs=wl_sb[li][:, kt, :],
                            start=(kt == 0), stop=(kt == ktn - 1))
                    g = gsbp.tile([128, dout], f16, tag="g")
                    nc.scalar.copy(g, gp)
                    g_sb.append(g)

                # --- S0: mains (+ C' for coupled chunks) ---
                mains = {}
                for ch in range(NCH):
                    for pt in range(ptn):
                        mp = mainp.tile([128, CH], f32, tag="mp",
                                        name=f"mp{ch}_{pt}")
                        for kt in range(ktn):
                            nc.tensor.matmul(
                                mp,
                                lhsT=wr_sb[li][:, kt, pt * 128:(pt + 1) * 128],
                                rhs=ain[ch][:, kt, :],
                                start=(kt == 0),
                                stop=(kt == ktn - 1 and ch >= 2))
                        if ch < 2:
                            its = [it for (it, j) in MT_BLOCKS if j == ch]
                            for ii, it in enumerate(its):
                                nc.tensor.matmul(
                                    mp,
                                    lhsT=g_sb[it][:, pt * 128:(pt + 1) * 128],
                                    rhs=mt_sb[:, _mt_block_id(it, ch), :],
                                    start=False, stop=(ii == len(its) - 1))
                        mains[(ch, pt)] = mp

                    # S1 immediately per chunk: raw16 = main + b (frees PSUM).
                    # Alternate ACT/DVE to balance engine load.
                    raw = rawp.tile([128, ptn, CH], f16, tag="raw",
                                    name=f"raw{ch}")
                    for pt in range(ptn):
                        if pt % 2 == 0:
                            nc.scalar.activation(
                                out=raw[:, pt, :], in_=mains[(ch, pt)],
                                func=AF.Identity,
                                bias=b_sb[li][:, pt:pt + 1], scale=1.0)
                        else:
                            nc.vector.tensor_scalar_add(
                                out=raw[:, pt, :], in0=mains[(ch, pt)],
                                scalar1=b_sb[li][:, pt:pt + 1])
                    mains[ch] = raw

                # --- S2: sq = (raw+b)^2 on DVE; S3: ss += ones.T @ sq ---
                sss = {}
                sqs = {}
                for ch in range(NCH):
                    raw = mains[ch]
                    sq = sqp.tile([128, ptn, CH], f16, tag="sq",
                                  name=f"sq{ch}")
                    for pt in range(ptn):
                        nc.vector.tensor_mul(
                            out=sq[:, pt, :], in0=raw[:, pt, :],
                            in1=raw[:, pt, :])
                    sqs[ch] = sq
                for ch in range(NCH):
                    ss = sspp.tile([128, CH], f32, tag="ss", name=f"ss{ch}")
                    for pt in range(ptn):
                        nc.tensor.matmul(ss, lhsT=ones128,
                                         rhs=sqs[ch][:, pt, :],
                                         start=(pt == 0), stop=(pt == ptn - 1))
                    sss[ch] = ss

                # --- S4: rsqrt of the broadcast sumsq, one wide ACT op ---
                sbs = {}
                for ch in range(NCH):
                    sb = sbcp.tile([128, CH], f16, tag="sbc", name=f"sb{ch}")
                    nc.scalar.activation(out=sb, in_=sss[ch],
                                         func=AF.Abs_reciprocal_sqrt)
                    sbs[ch] = sb

                # --- S7: apply scale (and bias where still missing) ---
                for ch in range(NCH):
                    raw = mains[ch]
                    sb = sbs[ch]
                    if not is_last:
                        if ch < 2:
                            aout = pa_sb[li]
                            asl = (slice(None), slice(None),
                                   slice(ch * CH, (ch + 1) * CH))
                        else:
                            anext = abp.tile([128, ptn, CH], f16, tag="ab",
                                             name=f"ab{ch}")
                            aout = anext
                            asl = (slice(None), slice(None), slice(0, CH))
                            ain[ch] = anext
                        for pt in range(ptn):
                            dst = aout[asl[0], pt, asl[2]]
                            nc.vector.tensor_mul(
                                out=dst, in0=raw[:, pt, :], in1=sb)
                        if ch < 2:
                            ain[ch] = pa_sb[li][:, :,
                                               ch * CH:(ch + 1) * CH]
                    else:
                        ost = ostp.tile([128, ptn, CH], f32, tag="ost",
                                        name=f"ost{ch}")
                        for pt in range(ptn):
                            # relu((raw+b)*s) = max(raw+b,0)*s since s>0
                            nc.vector.scalar_tensor_tensor(
                                out=ost[:, pt, :], in0=raw[:, pt, :],
                                scalar=0.0, in1=sb, op0=OP.max, op1=OP.mult)
                        nc.sync.dma_start(
                            out=out_r[:, :, ch * CH:(ch + 1) * CH], in_=ost)

    nc.compile()
    return nc


def _prep_inputs(flow_matrix, ws):
    """ws: list of (w_l, b_l, w_r) fp32. Returns list of 8 in_maps."""
    x = np.ascontiguousarray(flow_matrix.reshape(B * K, P))
    xt_full = np.ascontiguousarray(x.T.astype(np.float16))  # [P, 32768]

    # M.T packed nonzero blocks, fp16
    inv = np.zeros(K, np.float32)
    inv[1:] = 1.0 / np.arange(1, K, dtype=np.float32)
    mt_packed = np.zeros((128, len(MT_BLOCKS) * CH), np.float16)
    for bid, (it, jc) in enumerate(MT_BLOCKS):
        i0, j0 = it * 128, jc * CH
        blk = np.zeros((128, CH), np.float32)
        for pp in range(128):
            i = i0 + pp
            jj = np.arange(j0, j0 + CH)
            blk[pp] = np.where(jj > i, inv[jj], 0.0)
        mt_packed[:, bid * CH:(bid + 1) * CH] = blk.astype(np.float16)
    mt_zero = np.zeros_like(mt_packed)

    def pack_w(wt):  # [din, dout] -> [128, kt*dout] partition-major
        din, dout = wt.shape
        kt = din // 128
        return np.ascontiguousarray(
            wt.reshape(kt, 128, dout).transpose(1, 0, 2).reshape(128, -1)
            .astype(np.float16))

    base = {}
    for li, (w_l, b_l, w_r) in enumerate(ws):
        base[f"wr{li}"] = pack_w(w_r.T)
        base[f"wl{li}"] = pack_w(w_l.T)
        base[f"b{li}"] = np.ascontiguousarray(
            b_l.reshape(-1, 128).T.astype(np.float32))

    in_maps = []
    for c in range(N_CORES):
        m = dict(base)
        m["xt"] = np.ascontiguousarray(xt_full[:, c * RPC:(c + 1) * RPC])
        m["mt"] = mt_packed if c == 0 else mt_zero
        in_maps.append(m)
    return in_maps


def kernel(flow_matrix, w_l1, b_l1, w_r1, w_l2, b_l2, w_r2,
           w_l3, b_l3, w_r3, w_l4, b_l4, w_r4, _trace=False, _tmpdir=None):
    from concourse import bass_utils

    flow_matrix = np.asarray(flow_matrix, dtype=np.float32)
    ws = [(np.asarray(w_l1, np.float32), np.asarray(b_l1, np.float32),
           np.asarray(w_r1, np.float32)),
          (np.asarray(w_l2, np.float32), np.asarray(b_l2, np.float32),
           np.asarray(w_r2, np.float32)),
          (np.asarray(w_l3, np.float32), np.asarray(b_l3, np.float32),
           np.asarray(w_r3, np.float32)),
          (np.asarray(w_l4, np.float32), np.asarray(b_l4, np.float32),
           np.asarray(w_r4, np.float32))]

    if "nc" not in _CACHE:
        _CACHE["nc"] = _build_program()
    nc = _CACHE["nc"]

    in_maps = _prep_inputs(flow_matrix, ws)
    res = bass_utils.run_bass_kernel_spmd(
        nc, in_maps, core_ids=list(range(N_CORES)), trace=_trace,
        tmpdir=_tmpdir)

    y = np.empty((B * K, DOUT), np.float32)
    for c in range(N_CORES):
        out_c = res.results[c]["out"]  # [DOUT, RPC] fp32
        y[c * RPC:(c + 1) * RPC, :] = out_c.T
    _CACHE["last_exec_time_ns"] = res.exec_time_ns
    return (y.reshape(B, K, DOUT), 1)


if __name__ == "__main__":
    rng = np.random.default_rng(0)
    fm = rng.standard_normal((B, K, P)).astype(np.float32)
    args = []
    for (din, dout) in DIMS:
        s = 1.0 / np.sqrt(din)
        args += [rng.uniform(-s, s, (dout, din)).astype(np.float32),
                 rng.uniform(-s, s, dout).astype(np.float32),
                 rng.uniform(-s, s, (dout, din)).astype(np.float32)]
    y, _ = kernel(fm, *args)
    print("ok", y.shape, y.dtype)
